# revision 29
# baseline (speedup 1.0000x reference)
"""Trainium2 Bass kernel for ConditionalDeepConvexFlow forward_transform.

Returns (f = grad_x F(x), logdet of per-sample 32x32 Hessian) for B=1024
samples, data-parallel over 8 NeuronCores (128 samples/core).

Math (exact, no autodiff): with gsp(a) = a*Phi(a) + phi(a) (Phi/phi = std
normal CDF/PDF), gsp' = Phi, gsp'' = phi. Per sample:
  A_l = (h_l + bias_l) * s_l,  D_l = Phi(A_l)*s_l,  E_l = phi(A_l)*s_l^2
  grad chain: g2 = a*vout*D2, bv1 = V2^T g2, g1 = bv1*D1, bv0 = V1^T g1,
              g0 = bv0*D0, f = Wz0^T g0 + Wx1^T g1 + Wx2^T g2 + a*Wxout + b*x
  Jacobian prop: P1 = Wx1 + V1 (D0 . P0), P2 = Wx2 + V2 (D1 . P1), P0 = Wz0
  H = b*I + P2^T diag(a*vout*E2) P2 + P1^T diag(bv1*E1) P1
          + P0^T diag(bv0*E0) P0            (all coeffs >= 0 -> H is SPD)
  logdet via batched in-place LDL^T (samples on partitions).

Hardware-shaped constraints honored throughout:
 - DVE/ACT instructions have a single sync-wait slot: every such op may have
   at most ONE fresh cross-engine dependency. SBUF pools never close (no
   recycled multi-writer memory); only PSUM recycles, where the first
   toucher is always a matmul (large wait table).
 - HW-DGE DMAs fan out over up to 8 lane semaphores; any DMA whose direct
   consumer is DVE/ACT goes through single-lane SW DGE (gpsimd) instead.
"""

import numpy as np

B, DIM, DIMH, DIMC = 1024, 32, 1024, 512
NCORES = 8
BP = B // NCORES   # 128 samples per core
GS = 16            # samples per P-prop group
NG = BP // GS      # 8 groups
GW = GS * DIM      # 512 = P-prop moving width per group
NQ = BP // 4       # 32 sample-quads per core
SQ2PI = float(np.sqrt(2.0 * np.pi))
ISQ2 = float(1.0 / np.sqrt(2.0))

_CACHE = {}
LAST_RESULTS = None


def _split_multi_waits(bir_bytes):
    """walrus codegen allows one sync-wait slot per instruction; hoist extra
    waits onto inserted single-wait EventSemaphore instructions just before,
    on the same engine (pure added stalls, semantics preserved)."""
    import json
    d = json.loads(bir_bytes)
    cnt = 0
    for fn in d["functions"]:
        for blk in fn["blocks"]:
            out = []
            for inst in blk["instructions"]:
                si = inst.get("sync_info")
                waits = (si or {}).get("on_wait") or []
                if len(waits) > 1:
                    for w in waits[:-1]:
                        cnt += 1
                        out.append({
                            "debug": inst.get("debug", 0),
                            "engine": inst["engine"],
                            "ins": [],
                            "name": f"evw{cnt}-{inst['name']}",
                            "opcode": "EventSemaphore",
                            "outs": [],
                            "sync_info": {"on_update": [], "on_wait": [w]},
                        })
                    si["on_wait"] = [waits[-1]]
                out.append(inst)
            blk["instructions"] = out
    return json.dumps(d).encode()


def _bc(ap, dims):
    """AP keeping `ap`'s partition dim with explicit free dims [step,count]."""
    import concourse.bass as bass
    return bass.AP(tensor=ap.tensor, offset=ap.offset, ap=[list(ap.ap[0])] + dims)


def _build():
    import concourse.bass as bass
    import concourse.mybir as mybir
    import concourse.tile as tile
    from contextlib import ExitStack

    fp32 = mybir.dt.float32
    bf16 = mybir.dt.bfloat16
    AF = mybir.ActivationFunctionType
    OP = mybir.AluOpType

    nc = bass.Bass("TRN2")

    def inp(name, shape, dt=fp32):
        return nc.dram_tensor(name, shape, dt, kind="ExternalInput")

    xT = inp("xT", [DIM, BP]); xsm = inp("xsm", [BP, DIM]); cT = inp("cT", [DIMC, BP])
    wz0T = inp("wz0T", [DIM, DIMH]); wz0o = inp("wz0o", [DIMH, DIM])
    wc0T = inp("wc0T", [DIMC, DIMH])
    wx1T = inp("wx1T", [DIM, DIMH]); wx1o = inp("wx1o", [DIMH, DIM])
    wc1T = inp("wc1T", [DIMC, DIMH])
    wx2T = inp("wx2T", [DIM, DIMH]); wx2o = inp("wx2o", [DIMH, DIM])
    wc2T = inp("wc2T", [DIMC, DIMH])
    v1T_raw = inp("v1T_raw", [DIMH, DIMH]); v1o_raw = inp("v1o_raw", [DIMH, DIMH])
    v2T_raw = inp("v2T_raw", [DIMH, DIMH]); v2o_raw = inp("v2o_raw", [DIMH, DIMH])
    vout_raw = inp("vout_raw", [DIMH]); wxout_in = inp("wxout", [DIM])
    w0s_in = inp("w0s", [1]); w1s_in = inp("w1s", [1])
    eye_in = inp("eye", [DIM, GW], dt=bf16)
    BVN = ("z0b", "c0b", "a0b", "l0", "z1b", "x1b", "c1b", "a1b", "l1",
           "z2b", "x2b", "c2b", "a2b", "l2")
    bvs_in = inp("bvs", [len(BVN), DIMH])

    f_out = nc.dram_tensor("f_out", [BP, DIM], fp32, kind="ExternalOutput")
    ld_out = nc.dram_tensor("ld_out", [BP], fp32, kind="ExternalOutput")

    # gram relayout scratch, layout (a, j, q, j')
    gscr = nc.dram_tensor("gram_scratch", [4, DIM, NQ, DIM], fp32, kind="Internal")

    KT = DIMH // 128  # 8 hidden tiles
    CK = DIMC // 128  # 4 cond tiles

    with ExitStack() as ctx:
        tc = ctx.enter_context(tile.TileContext(nc))
        # every SBUF pool is kernel-lifetime: no SBUF memory recycling
        const = ctx.enter_context(tc.tile_pool(name="const", bufs=1))
        wts = ctx.enter_context(tc.tile_pool(name="wts", bufs=1))
        work = ctx.enter_context(tc.tile_pool(name="work", bufs=1))
        grp = ctx.enter_context(tc.tile_pool(name="grp", bufs=1))
        keep = ctx.enter_context(tc.tile_pool(name="keep", bufs=1))
        sc = ctx.enter_context(tc.tile_pool(name="scratch", bufs=2))
        raw = ctx.enter_context(tc.tile_pool(name="raw", bufs=2))

        def TL(pool, shape, tag, dt=fp32, bufs=None):
            return pool.tile(shape, dt, tag=tag, name=tag, bufs=bufs)

        # ---------------- staging: DMAs ----------------
        xT_sb = TL(const, [DIM, BP], "xT")
        nc.gpsimd.dma_start(out=xT_sb, in_=xT[:, :])
        xsm_sb = TL(const, [BP, DIM], "xsm")
        nc.gpsimd.dma_start(out=xsm_sb, in_=xsm[:, :])
        cT_sb = [TL(const, [128, BP], f"cT{k}") for k in range(CK)]
        for k in range(CK):
            nc.gpsimd.dma_start(out=cT_sb[k], in_=cT[k * 128:(k + 1) * 128, :])
        eye_sb = TL(const, [DIM, GW], "eye", dt=bf16)
        nc.gpsimd.dma_start(out=eye_sb, in_=eye_in[:, :])

        aP = TL(const, [128, 1], "aP")
        nc.gpsimd.dma_start(out=aP, in_=w1s_in[:].rearrange("(a b) -> a b", a=1).to_broadcast((128, 1)))
        bP = TL(const, [128, 1], "bP")
        nc.gpsimd.dma_start(out=bP, in_=w0s_in[:].rearrange("(a b) -> a b", a=1).to_broadcast((128, 1)))
        wxo_sb = TL(const, [1, DIM], "wxo")
        nc.gpsimd.dma_start(out=wxo_sb, in_=wxout_in[:].rearrange("(a j) -> a j", a=1))

        nbv = len(BVN)
        bvs_sb = TL(const, [128, nbv * KT], "bvs_sb")
        nc.gpsimd.dma_start(
            out=bvs_sb,
            in_=bvs_in[:, :].rearrange("n (t p) -> p (n t)", p=128))
        vb = {nm: bvs_sb[:, i * KT:(i + 1) * KT] for i, nm in enumerate(BVN)}
        avout = TL(const, [128, KT], "avout")
        nc.gpsimd.dma_start(out=avout, in_=vout_raw[:].rearrange("(t p) -> p t", p=128))

        wx1T_sb = TL(wts, [DIM, DIMH], "wx1T")
        nc.gpsimd.dma_start(out=wx1T_sb, in_=wx1T[:, :])
        wx2T_sb = TL(wts, [DIM, DIMH], "wx2T")
        nc.gpsimd.dma_start(out=wx2T_sb, in_=wx2T[:, :])
        wz0T_sb = TL(wts, [DIM, DIMH], "wz0T")
        nc.gpsimd.dma_start(out=wz0T_sb, in_=wz0T[:, :])
        wz0o_sb = [TL(wts, [128, DIM], f"wz0o{k}") for k in range(KT)]
        wx1o_sb = [TL(wts, [128, DIM], f"wx1o{k}") for k in range(KT)]
        wx2o_sb = [TL(wts, [128, DIM], f"wx2o{k}") for k in range(KT)]
        for k in range(KT):
            sl = slice(k * 128, (k + 1) * 128)
            nc.gpsimd.dma_start(out=wz0o_sb[k], in_=wz0o[sl, :])
            nc.sync.dma_start(out=wx1o_sb[k], in_=wx1o[sl, :])
            nc.sync.dma_start(out=wx2o_sb[k], in_=wx2o[sl, :])

        # ---------------- staging: compute ----------------
        def softplus_(out, in_):
            # no native softplus table: ln(1 + exp(x)) via Exp then Ln(x+1)
            nc.scalar.activation(out=in_, in_=in_, func=AF.Exp)
            nc.scalar.activation(out=out, in_=in_, func=AF.Ln, bias=1.0)

        # ACT warmup: absorb the const-AP staging DMA semaphore first
        dummy = TL(const, [1, 1], "dummy")
        ca0 = nc.const_aps.scalar_like(0.0, dummy[0:1, 0:1])
        nc.scalar.activation(out=dummy, in_=ca0, func=AF.Identity)

        ones_sb = TL(const, [1, 128], "ones")
        nc.vector.memset(ones_sb, 1.0)
        softplus_(aP, aP)
        softplus_(bP, bP)
        awxo = TL(const, [1, DIM], "awxo")
        nc.scalar.activation(out=awxo, in_=wxo_sb, func=AF.Identity, scale=aP[0:1, 0:1])

        lay = []
        for li, (bias_parts, logs) in enumerate((
                (("z0b", "c0b", "a0b"), "l0"),
                (("z1b", "x1b", "c1b", "a1b"), "l1"),
                (("z2b", "x2b", "c2b", "a2b"), "l2"))):
            s = TL(const, [128, KT], f"s{li}")
            nc.scalar.activation(out=s, in_=vb[logs], func=AF.Exp)
            bsum = TL(const, [128, KT], f"bsum{li}")
            nc.vector.tensor_add(bsum, vb[bias_parts[0]], vb[bias_parts[1]])
            for extra in bias_parts[2:]:
                nc.vector.tensor_add(bsum, bsum, vb[extra])
            bt = TL(const, [128, KT], f"bt{li}")
            nc.vector.tensor_mul(bt, bsum, s)
            s2r = TL(const, [128, KT], f"s2r{li}")
            nc.vector.tensor_scalar_mul(s2r, s, ISQ2)
            bt2r = TL(const, [128, KT], f"bt2r{li}")
            nc.vector.tensor_scalar_mul(bt2r, bt, ISQ2)
            sh = TL(const, [128, KT], f"sh{li}")
            nc.vector.tensor_scalar_mul(sh, s, 0.5)
            ec = TL(const, [128, KT], f"ec{li}")
            nc.vector.tensor_mul(ec, s, s)
            nc.vector.tensor_scalar_mul(ec, ec, 1.0 / SQ2PI)
            lay.append(dict(s=s, bt=bt, s2r=s2r, bt2r=bt2r, sh=sh, ec=ec))

        softplus_(avout, avout)
        nc.vector.tensor_scalar_mul(avout, avout, 1.0 / DIMH)
        nc.vector.tensor_scalar_mul(avout, avout, aP)

        # V^T weights: stream raw fp32 chunks through SW DGE, softplus -> bf16
        v1T_sb = [TL(wts, [128, DIMH], f"v1T{k}", dt=bf16) for k in range(KT)]
        v2T_sb = [TL(wts, [128, DIMH], f"v2T{k}", dt=bf16) for k in range(KT)]
        for dst, hnd in ((v1T_sb, v1T_raw), (v2T_sb, v2T_raw)):
            for k in range(KT):
                r = TL(raw, [128, DIMH], "vraw_in")
                nc.gpsimd.dma_start(out=r, in_=hnd[k * 128:(k + 1) * 128, :])
                softplus_(dst[k], r)
                nc.vector.tensor_scalar_mul(dst[k], dst[k], 1.0 / DIMH)
        wx1Tb = TL(wts, [DIM, DIMH], "wx1Tb", dt=bf16)
        nc.vector.tensor_copy(wx1Tb, wx1T_sb)
        wx2Tb = TL(wts, [DIM, DIMH], "wx2Tb", dt=bf16)
        nc.vector.tensor_copy(wx2Tb, wx2T_sb)

        # ======================= forward =======================
        wc_sb = [TL(work, [128, DIMH], f"wc_{k}") for k in range(CK)]

        def load_wc(handle):
            for k in range(CK):
                nc.gpsimd.dma_start(out=wc_sb[k], in_=handle[k * 128:(k + 1) * 128, :])

        with tc.tile_pool(name="pfwd", bufs=3, space="PSUM") as pfwd:
            def fwd_layer(li, z_prev, vT, wxT, wc_h, want_z):
                L = lay[li]
                load_wc(wc_h)
                Z = [TL(work, [128, BP], f"Z{li}_{m}", dt=bf16) for m in range(KT)] if want_z else None
                D = [TL(work, [128, BP], f"D{li}_{m}") for m in range(KT)]
                E = [TL(work, [128, BP], f"E{li}_{m}") for m in range(KT)]
                for m in range(KT):
                    msl = slice(m * 128, (m + 1) * 128)
                    hp = TL(pfwd, [128, BP], "hp")
                    first = True
                    if vT is not None:
                        for k in range(KT):
                            nc.tensor.matmul(hp, lhsT=vT[k][:, msl], rhs=z_prev[k],
                                             start=first, stop=False)
                            first = False
                    nc.tensor.matmul(hp, lhsT=wxT[:, msl], rhs=xT_sb,
                                     start=first, stop=False)
                    for k in range(CK):
                        nc.tensor.matmul(hp, lhsT=wc_sb[k][:, msl], rhs=cT_sb[k],
                                         start=False, stop=(k == CK - 1))
                    mc = slice(m, m + 1)
                    R = TL(sc, [128, BP], "actR")
                    nc.scalar.activation(out=R, in_=hp, func=AF.Erf,
                                         bias=L["bt2r"][:, mc], scale=L["s2r"][:, mc])
                    SQ = TL(sc, [128, BP], "actSQ")
                    nc.scalar.activation(out=SQ, in_=hp, func=AF.Square,
                                         bias=L["bt2r"][:, mc], scale=L["s2r"][:, mc])
                    X2 = TL(sc, [128, BP], "actX2", bufs=3)
                    nc.scalar.activation(out=X2, in_=SQ, func=AF.Exp, scale=-1.0)
                    nc.vector.tensor_scalar(D[m], R, L["sh"][:, mc], L["sh"][:, mc],
                                            op0=OP.mult, op1=OP.add)
                    nc.vector.tensor_scalar_mul(E[m], X2, L["ec"][:, mc])
                    if want_z:
                        G = TL(sc, [128, BP], "actG", bufs=3)
                        nc.scalar.activation(out=G, in_=hp, func=AF.Gelu,
                                             bias=L["bt"][:, mc], scale=L["s"][:, mc])
                        nc.vector.tensor_scalar_mul(X2, X2, 1.0 / SQ2PI)
                        nc.vector.tensor_add(Z[m], G, X2)
                return Z, D, E

            Z0, D0, E0 = fwd_layer(0, None, None, wz0T_sb, wc0T, True)
            Z1, D1, E1 = fwd_layer(1, Z0, v1T_sb, wx1T_sb, wc1T, True)
            _, D2, E2 = fwd_layer(2, Z1, v2T_sb, wx2T_sb, wc2T, False)

        # ======================= backward + f =======================
        g2 = D2  # in-place: D2 not needed past g2
        for m in range(KT):
            nc.vector.tensor_scalar_mul(g2[m], D2[m], avout[:, m:m + 1])

        with tc.tile_pool(name="pbv", bufs=1, space="PSUM") as pbv:
            def bwd(vo_raw_h, g, onm):
                # one PSUM bank per output slice: matmul start=True clears the
                # whole bank, so slices must not share banks across groups
                bvp = [TL(pbv, [128, 128], f"bvp{m}") for m in range(KT)]
                for k in range(KT):
                    r = TL(raw, [128, DIMH], "vraw_in")
                    nc.gpsimd.dma_start(out=r, in_=vo_raw_h[k * 128:(k + 1) * 128, :])
                    softplus_(r, r)
                    for m in range(KT):
                        nc.tensor.matmul(bvp[m],
                                         lhsT=r[:, m * 128:(m + 1) * 128], rhs=g[k],
                                         start=(k == 0), stop=(k == KT - 1),
                                         skip_group_check=True)
                bv = [TL(work, [128, BP], f"bv{onm}_{m}") for m in range(KT)]
                for m in range(KT):
                    nc.scalar.activation(out=bv[m], in_=bvp[m],
                                         func=AF.Identity, scale=1.0 / DIMH)
                return bv

            bv1 = bwd(v2o_raw, g2, "1")
            g1 = [TL(work, [128, BP], f"g1_{m}") for m in range(KT)]
            for m in range(KT):
                nc.vector.tensor_mul(g1[m], bv1[m], D1[m])
            bv0 = bwd(v1o_raw, g1, "0")
            g0 = [TL(work, [128, BP], f"g0_{m}") for m in range(KT)]
            for m in range(KT):
                nc.vector.tensor_mul(g0[m], bv0[m], D0[m])

        with tc.tile_pool(name="pf", bufs=1, space="PSUM") as pf:
            fp_ = TL(pf, [BP, DIM], "fpsum")
            nc.tensor.matmul(fp_, lhsT=ones_sb, rhs=awxo, start=True, stop=False)
            for g, w in ((g0, wz0o_sb), (g1, wx1o_sb), (g2, wx2o_sb)):
                for k in range(KT):
                    last = g is g2 and k == KT - 1
                    nc.tensor.matmul(fp_, lhsT=g[k], rhs=w[k], start=False, stop=last)
            fsb = TL(sc, [BP, DIM], "fsb")
            touch = TL(sc, [1, 1], "touch")
            nc.vector.tensor_copy(touch, xsm_sb[0:1, 0:1])  # absorb xsm DMA sem
            nc.vector.tensor_scalar_mul(fsb, xsm_sb, bP)    # fresh: ACT(bP)
            nc.vector.tensor_add(fsb, fsb, fp_)             # fresh: PE(fp_)
            nc.sync.dma_start(out=f_out[:, :], in_=fsb)

        # ---------------- Hessian weights ----------------
        sqw1r, sqw2 = E1, E2  # computed in place
        # w0v columns written permuted to (a, q) order so GramL0's output
        # partitions match the H-pipeline layout p = a*32+q (sample 4q+a)
        w0v = [TL(work, [128, BP], f"w0v_{m}", dt=bf16) for m in range(KT)]
        for m in range(KT):
            ov = _bc(w0v[m][:, :], [[1, NQ], [NQ, 4]])
            iv0 = _bc(bv0[m][:, :], [[4, NQ], [1, 4]])
            iv1 = _bc(E0[m][:, :], [[4, NQ], [1, 4]])
            nc.vector.tensor_tensor(out=ov, in0=iv0, in1=iv1, op=OP.mult)
            nc.vector.tensor_scalar_mul(sqw2[m], sqw2[m], avout[:, m:m + 1])
            nc.scalar.activation(out=sqw2[m], in_=sqw2[m], func=AF.Sqrt)
            # drain uses P1 from PSUM directly, so the weight is sqrt(bv1*E1)
            nc.vector.tensor_mul(sqw1r[m], bv1[m], E1[m])
            nc.scalar.activation(out=sqw1r[m], in_=sqw1r[m], func=AF.Sqrt)

        H = TL(keep, [128, DIM, DIM], "H")

        # ---------------- Gram level 0 (O-trick, sample-major) ----------------
        with tc.tile_pool(name="ph0", bufs=1, space="PSUM") as ph0:
            h0p = TL(ph0, [128, DIMH], "h0p")
            for k in range(KT):
                ot = TL(sc, [128, DIM, DIM], "otile", dt=bf16, bufs=2)
                wj = _bc(wz0o_sb[k][:, :], [list(wz0o_sb[k][:, :].ap[1]), [0, DIM]])
                wj2 = _bc(wz0o_sb[k][:, :], [[0, DIM], list(wz0o_sb[k][:, :].ap[1])])
                nc.vector.tensor_tensor(out=ot[:, :, :], in0=wj, in1=wj2, op=OP.mult)
                o2 = ot[:, :, :].rearrange("p a b -> p (a b)")
                for ch in range(2):
                    nc.tensor.matmul(h0p[:, ch * 512:(ch + 1) * 512],
                                     lhsT=w0v[k],
                                     rhs=o2[:, ch * 512:(ch + 1) * 512],
                                     start=(k == 0), stop=(k == KT - 1),
                                     skip_group_check=True)
            nc.vector.tensor_copy(H[:, :, :].rearrange("p a b -> p (a b)"), h0p[:, :])

        # ---------------- P-prop + Gram 1/2, per sample-group ----------------
        gall = TL(keep, [128, NQ * DIM], "gall")  # (a*32+j) x (q*32+j')
        with tc.tile_pool(name="ppp", bufs=3, space="PSUM") as ppp, \
             tc.tile_pool(name="pgr", bufs=2, space="PSUM") as pgr:
            for g in range(NG):
                i0 = g * GS
                isl = slice(i0, i0 + GS)
                q0 = [TL(grp, [128, GS, DIM], f"q0_{m}", dt=bf16, bufs=2) for m in range(KT)]
                q1 = [TL(grp, [128, GS, DIM], f"q1_{m}", dt=bf16) for m in range(KT)]
                s1 = [TL(grp, [128, GS, DIM], f"s1_{m}", dt=bf16) for m in range(KT)]
                s2 = [TL(grp, [128, GS, DIM], f"s2_{m}", dt=bf16) for m in range(KT)]
                for m in range(KT):
                    d_b = _bc(D0[m][:, isl], [list(D0[m][:, isl].ap[1]), [0, DIM]])
                    w_b = _bc(wz0o_sb[m][:, :], [[0, GS], list(wz0o_sb[m][:, :].ap[1])])
                    nc.vector.tensor_tensor(out=q0[m][:, :, :], in0=d_b, in1=w_b, op=OP.mult)

                for m in range(KT):
                    msl = slice(m * 128, (m + 1) * 128)
                    pp = TL(ppp, [128, GW], "pp")
                    for k in range(KT):
                        r3 = q0[k][:, :, :].rearrange("p b j -> p (b j)")
                        nc.tensor.matmul(pp, lhsT=v1T_sb[k][:, msl],
                                         rhs=r3, start=(k == 0), stop=False)
                    nc.tensor.matmul(pp, lhsT=wx1Tb[:, msl],
                                     rhs=eye_sb[:, :], start=False, stop=True)
                    ppv = pp[:, :].rearrange("p (b j) -> p b j", j=DIM)
                    d_b = _bc(D1[m][:, isl], [list(D1[m][:, isl].ap[1]), [0, DIM]])
                    w_b = _bc(sqw1r[m][:, isl], [list(sqw1r[m][:, isl].ap[1]), [0, DIM]])
                    nc.vector.tensor_tensor(out=q1[m][:, :, :], in0=ppv, in1=d_b, op=OP.mult)
                    nc.vector.tensor_tensor(out=s1[m][:, :, :], in0=ppv, in1=w_b, op=OP.mult)

                for m in range(KT):
                    msl = slice(m * 128, (m + 1) * 128)
                    pp = TL(ppp, [128, GW], "pp")
                    for k in range(KT):
                        r3 = q1[k][:, :, :].rearrange("p b j -> p (b j)")
                        nc.tensor.matmul(pp, lhsT=v2T_sb[k][:, msl],
                                         rhs=r3, start=(k == 0), stop=False)
                    nc.tensor.matmul(pp, lhsT=wx2Tb[:, msl],
                                     rhs=eye_sb[:, :], start=False, stop=True)
                    ppv = pp[:, :].rearrange("p (b j) -> p b j", j=DIM)
                    w_b = _bc(sqw2[m][:, isl], [list(sqw2[m][:, isl].ap[1]), [0, DIM]])
                    nc.vector.tensor_tensor(out=s2[m][:, :, :], in0=ppv, in1=w_b, op=OP.mult)

                for ql in range(GS // 4):
                    q = g * (GS // 4) + ql
                    qsl = slice(ql * 128, (ql + 1) * 128)
                    gp = TL(pgr, [128, 128], "gp")
                    for k in range(KT):
                        sv = s1[k][:, :, :].rearrange("p b j -> p (b j)")[:, qsl]
                        nc.tensor.matmul(gp, lhsT=sv, rhs=sv, start=(k == 0), stop=False)
                    for k in range(KT):
                        sv = s2[k][:, :, :].rearrange("p b j -> p (b j)")[:, qsl]
                        nc.tensor.matmul(gp, lhsT=sv, rhs=sv, start=False, stop=(k == KT - 1))
                    # compact the 4 diagonal 32x32 blocks into gall columns
                    for a in range(4):
                        if a % 2 == 0:
                            nc.vector.tensor_copy(
                                gall[32 * a:32 * a + 32, q * DIM:(q + 1) * DIM],
                                gp[32 * a:32 * a + 32, 32 * a:32 * a + 32])
                        else:
                            nc.scalar.copy(
                                out=gall[32 * a:32 * a + 32, q * DIM:(q + 1) * DIM],
                                in_=gp[32 * a:32 * a + 32, 32 * a:32 * a + 32])

        # ---------------- assemble H and batched LDL^T ----------------
        # gall[(a,j), (q,j')] -> DRAM (a,j,q,j') -> per-a gather into Hg[(q,a),(j,j')]
        nc.gpsimd.dma_start(out=gscr[:, :, :, :].rearrange("a j q k -> (a j) (q k)"),
                          in_=gall)
        Hg = TL(keep, [128, DIM, DIM], "Hg")
        for a in range(4):
            sbase = gscr[a, :, :, :]  # dims (j, q, k); need iteration (q, j, k)
            srcap = bass.AP(tensor=sbase.tensor, offset=sbase.offset,
                            ap=[[DIM, NQ], [NQ * DIM, DIM], [1, DIM]])
            nc.gpsimd.dma_start(out=Hg[32 * a:32 * (a + 1), :, :], in_=srcap)
        for a in range(4):
            sl = slice(32 * a, 32 * (a + 1))
            nc.vector.tensor_tensor(out=H[sl, :, :], in0=H[sl, :, :],
                                    in1=Hg[sl, :, :], op=OP.add)

        d00 = H[:, 0, 0:1]
        est = H[:, 0, :].ap[1][0]  # element step of last dim
        diag = bass.AP(tensor=d00.tensor, offset=d00.offset,
                       ap=[list(d00.ap[0]), [(DIM + 1) * est, DIM]])
        nc.vector.tensor_scalar_add(diag, diag, bP)

        stile = TL(keep, [128, DIM], "chol_s")
        outer = TL(keep, [128, DIM, DIM], "chol_o")
        for k in range(DIM - 1):
            n = DIM - 1 - k
            rcp1 = TL(sc, [128, 1], "chol_r")
            nc.vector.reciprocal(rcp1, H[:, k, k:k + 1])
            row = H[:, k, k + 1:]
            nc.vector.tensor_scalar_mul(stile[:, :n], row, rcp1)
            s_b = _bc(stile[:, :n], [list(stile[:, :n].ap[1]), [0, n]])
            r_b = _bc(row, [[0, n], list(row.ap[1])])
            nc.vector.tensor_tensor(out=outer[:, :n, :n], in0=s_b, in1=r_b, op=OP.mult)
            sub = H[:, k + 1:, k + 1:]
            nc.vector.tensor_sub(sub, sub, outer[:, :n, :n])

        lnv = TL(keep, [128, DIM], "lnv")
        ld = TL(keep, [128, 1], "ld")
        nc.scalar.activation(out=lnv, in_=diag, func=AF.Ln, accum_out=ld)
        ldo = ld_out[:]
        ldperm = bass.AP(tensor=ldo.tensor, offset=ldo.offset,
                         ap=[[1, 4], [4, NQ], [1, 1]])
        nc.sync.dma_start(out=ldperm, in_=ld[:, 0:1])

    return nc


def _stage_inputs(inputs):
    """Build the 8 per-core input maps from full inputs."""
    import ml_dtypes
    ct = lambda a: np.ascontiguousarray(a, dtype=np.float32)
    x = inputs["x"]; c = inputs["c"]
    shared = {
        "wz0T": ct(inputs["Wz0_w"].T), "wz0o": ct(inputs["Wz0_w"]),
        "wc0T": ct(inputs["Wc0_w"].T),
        "wx1T": ct(inputs["Wx1_w"].T), "wx1o": ct(inputs["Wx1_w"]),
        "wc1T": ct(inputs["Wc1_w"].T),
        "wx2T": ct(inputs["Wx2_w"].T), "wx2o": ct(inputs["Wx2_w"]),
        "wc2T": ct(inputs["Wc2_w"].T),
        "v1T_raw": ct(inputs["Wz1_w"].T), "v1o_raw": ct(inputs["Wz1_w"]),
        "v2T_raw": ct(inputs["Wz2_w"].T), "v2o_raw": ct(inputs["Wz2_w"]),
        "vout_raw": ct(inputs["Wzout_w"][0]), "wxout": ct(inputs["Wxout_w"][0]),
        "w0s": ct(inputs["w0"]), "w1s": ct(inputs["w1"]),
        "eye": np.ascontiguousarray(
            np.tile(np.eye(DIM), (1, GW // DIM)).astype(ml_dtypes.bfloat16)),
        "bvs": ct(np.stack([
            inputs["Wz0_b"], inputs["Wc0_b"], inputs["an0_b"], inputs["an0_logs"],
            inputs["Wz1_b"], inputs["Wx1_b"], inputs["Wc1_b"], inputs["an1_b"], inputs["an1_logs"],
            inputs["Wz2_b"], inputs["Wx2_b"], inputs["Wc2_b"], inputs["an2_b"], inputs["an2_logs"],
        ])),
    }
    in_maps = []
    for ci in range(NCORES):
        sl = slice(ci * BP, (ci + 1) * BP)
        m = dict(shared)
        m["xT"] = ct(x[sl].T)
        m["xsm"] = ct(x[sl])
        m["cT"] = ct(c[sl].T)
        in_maps.append(m)
    return in_maps


def kernel(**inputs):
    global LAST_RESULTS
    import os
    from concourse.bass_utils import run_bass_kernel_spmd

    if "nc" not in _CACHE:
        nc0 = _build()
        orig = nc0.to_json_bytes
        nc0.to_json_bytes = lambda: _split_multi_waits(orig())
        _CACHE["nc"] = nc0
    nc = _CACHE["nc"]
    in_maps = _stage_inputs(inputs)
    want_trace = bool(int(os.environ.get("KERNEL_TRACE", "0")))
    try:
        res = run_bass_kernel_spmd(nc, in_maps, core_ids=list(range(NCORES)),
                                   trace=want_trace)
    except ModuleNotFoundError:
        # axon NTFF profile hook unavailable on this image
        res = run_bass_kernel_spmd(nc, in_maps, core_ids=list(range(NCORES)),
                                   trace=False)
    LAST_RESULTS = res
    f = np.concatenate([r["f_out"] for r in res.results], axis=0)
    ld = np.concatenate([r["ld_out"] for r in res.results], axis=0)
    return f.astype(np.float32), ld.astype(np.float32)
